# revision 1
# baseline (speedup 1.0000x reference)
"""Trainium2 Bass kernel for per-view cross-attention.

Reference computation (per view v of 1024, S=64 samples, D=256):
  qp = q @ Wq.T + pe ; kp = k @ Wk.T + pe ; vp = v @ Wv.T + pe
  attn = softmax(qp @ kp.T / sqrt(D))
  x = gelu(attn @ vp @ Wo.T + bo) + q
Sharding: data-parallel over the 1024 views across 8 cores (128 views each).

On-chip layout strategy: everything is kept in "transposed" space [D, rows]
(rows = view*64+s) so that the contraction dim D lands on SBUF partitions
without any on-chip input transposes. The host pre-transposes q/k/v shards to
[D, rows] (free: numpy) and post-transposes the [D, rows] output back.
v additionally needs its projected form in natural [row, D] layout for the
attn@v matmul; that drops out naturally by using vT as the matmul stationary.
"""

import sys
import os

for p in ("/opt/trn_rl_repo",):
    if p not in sys.path and os.path.isdir(p):
        sys.path.insert(0, p)

import numpy as np

V, S, D = 1024, 64, 256
N_CORES = 8
VC = V // N_CORES          # views per core
ROWS = VC * S              # 8192 rows per core
R = 512                    # rows per supertile (8 views)
NST = ROWS // R            # supertiles per core
NV = R // S                # views per supertile
GELU_GROUP = 4             # supertiles per gelu flush (ACT table amortization)
PROJ_BUFS = 3
SM_BUFS = 3
PS_S_BUFS = 1
PS_T_BUFS = 1
PS_A_BUFS = 3
PS_B_BUFS = 3
LD_BUFS = 3
SCALE = 1.0 / np.sqrt(np.float32(D)).astype(np.float32)

_CACHE = {}


def _make_posenc(d_hid, n_samples):
    pos = np.arange(n_samples, dtype=np.float64)[:, None]
    j = np.arange(d_hid)[None, :]
    angle = pos / np.power(10000.0, 2.0 * (j // 2) / d_hid)
    table = np.where(j % 2 == 0, np.sin(angle), np.cos(angle))
    return table.astype(np.float32)  # [S, D]


def _build(rows=ROWS, stage=99):
    import concourse.bass as bass
    import concourse.mybir as mybir
    import concourse.tile as tile
    from concourse.tile import add_dep_helper
    from concourse import bacc
    from contextlib import ExitStack

    fp32 = mybir.dt.float32
    f32r = mybir.dt.float32r
    bf16 = mybir.dt.bfloat16
    AF = mybir.ActivationFunctionType
    ALU = mybir.AluOpType
    n_st = rows // R

    nc = bacc.Bacc(None, target_bir_lowering=False)

    qT_d = nc.dram_tensor("qT", [D, rows], f32r, kind="ExternalInput")
    kT_d = nc.dram_tensor("kT", [D, rows], f32r, kind="ExternalInput")
    vT_d = nc.dram_tensor("vT", [D, rows], f32r, kind="ExternalInput")
    wq_d = nc.dram_tensor("WqT", [D, D], f32r, kind="ExternalInput")
    wk_d = nc.dram_tensor("WkT", [D, D], f32r, kind="ExternalInput")
    wv_d = nc.dram_tensor("WvT", [D, D], f32r, kind="ExternalInput")
    wo_d = nc.dram_tensor("WoT", [D, D], f32r, kind="ExternalInput")
    bo_d = nc.dram_tensor("bo", [D], fp32, kind="ExternalInput")
    pet_d = nc.dram_tensor("peT_rep", [D, R], fp32, kind="ExternalInput")
    pe_d = nc.dram_tensor("pe_nat", [S, D], f32r, kind="ExternalInput")
    e2_d = nc.dram_tensor("E2", [S, 128], f32r, kind="ExternalInput")
    id_d = nc.dram_tensor("I128", [128, 128], fp32, kind="ExternalInput")
    out_d = nc.dram_tensor("outT", [D, rows], fp32, kind="ExternalOutput")

    def r3(ap):  # [D, X] dram -> [128, 2, X] partition view
        return ap.rearrange("(kc p) r -> p kc r", p=128)

    with tile.TileContext(nc) as tc, ExitStack() as ctx:
        const = ctx.enter_context(tc.tile_pool(name="const", bufs=1))
        ld = ctx.enter_context(tc.tile_pool(name="ld", bufs=LD_BUFS))
        proj = ctx.enter_context(tc.tile_pool(name="proj", bufs=PROJ_BUFS))
        sm = ctx.enter_context(tc.tile_pool(name="sm", bufs=SM_BUFS))
        psA = ctx.enter_context(tc.tile_pool(name="psA", bufs=PS_A_BUFS, space="PSUM"))
        psB = ctx.enter_context(tc.tile_pool(name="psB", bufs=PS_B_BUFS, space="PSUM"))
        psS = ctx.enter_context(tc.tile_pool(name="psS", bufs=PS_S_BUFS, space="PSUM"))
        psT = ctx.enter_context(tc.tile_pool(name="psT", bufs=PS_T_BUFS, space="PSUM"))
        stg = ctx.enter_context(tc.tile_pool(name="stg", bufs=GELU_GROUP + 1))

        wq = const.tile([128, 2, D], f32r)
        wk = const.tile([128, 2, D], f32r)
        wv = const.tile([128, 2, D], f32r)
        wo = const.tile([128, 2, D], f32r)
        nc.sync.dma_start(wq, r3(wq_d[:]))
        nc.sync.dma_start(wk, r3(wk_d[:]))
        nc.sync.dma_start(wv, r3(wv_d[:]))
        nc.sync.dma_start(wo, r3(wo_d[:]))
        pet = const.tile([128, 2, R], fp32)
        nc.sync.dma_start(pet, r3(pet_d[:]))
        pe_sb = const.tile([S, D], f32r)
        nc.sync.dma_start(pe_sb, pe_d[:])
        e2 = const.tile([S, 128], f32r)
        nc.sync.dma_start(e2, e2_d[:])
        i128 = const.tile([128, 128], fp32)
        nc.sync.dma_start(i128, id_d[:])
        bo_sb = const.tile([128, 2], fp32)
        nc.sync.dma_start(bo_sb, bo_d.rearrange("(kc p) -> p kc", p=128))

        pending = []
        last_gelu = None
        last_exp = None
        for st in range(n_st):
            rs = slice(st * R, (st + 1) * R)
            qt = ld.tile([128, 2, R], f32r, tag="qt", bufs=GELU_GROUP + 2)
            kt = ld.tile([128, 2, R], f32r, tag="kt")
            vt = ld.tile([128, 2, R], f32r, tag="vt")
            nc.sync.dma_start(qt, r3(qT_d[:])[:, :, rs])
            nc.sync.dma_start(kt, r3(kT_d[:])[:, :, rs])
            nc.sync.dma_start(vt, r3(vT_d[:])[:, :, rs])

            # ---- projections into transposed space: xpT[dout, row] ----
            qpT = proj.tile([128, 2, R], fp32, tag="qpT")
            kpT = proj.tile([128, 2, R], fp32, tag="kpT")
            for w_sb, x_sb, o_sb in ((wq, qt, qpT), (wk, kt, kpT)):
                for mc in range(2):
                    ps = psA.tile([128, R], fp32, tag="psA", name="ps_proj")
                    for kc in range(2):
                        nc.tensor.matmul(
                            ps,
                            w_sb[:, kc, mc * 128:(mc + 1) * 128],
                            x_sb[:, kc, :],
                            start=(kc == 0),
                            stop=(kc == 1),
                        )
                    # evacuate PSUM fused with positional-encoding add
                    nc.vector.tensor_add(
                        out=o_sb[:, mc, :], in0=ps, in1=pet[:, mc, :]
                    )

            if stage <= 1:
                nc.sync.dma_start(r3(out_d[:])[:, :, rs], qpT)
                continue
            # ---- vp in natural [row, dout] layout (vT as stationary) ----
            vp = proj.tile([128, 4, D], fp32, tag="vp")
            for g in range(4):
                psv = psB.tile([128, D], fp32, tag="psB", name="ps_vp")
                for kc in range(2):
                    nc.tensor.matmul(
                        psv,
                        vt[:, kc, g * 128:(g + 1) * 128],
                        wv[:, kc, :],
                        start=(kc == 0),
                        stop=False,
                    )
                # pe add folded in as a matmul: E2.T @ pe = pe tiled over rows
                nc.tensor.matmul(psv, e2, pe_sb, start=False, stop=True)
                nc.scalar.copy(out=vp[:, g, :], in_=psv)

            if stage <= 2:
                nc.sync.dma_start(r3(out_d[:])[:, :, rs], vp.rearrange("p a b -> p (a b)")[:, None, :].rearrange("p o (a b) -> p (o a) b", a=2))
                continue
            # ---- scores: per view [64,64], packed [128(2 views), 4, 64] ----
            scps = psS.tile([128, 4, S], fp32, tag="scores")
            for v in range(NV):
                g, h = v // 2, v % 2
                for dc in range(2):
                    nc.tensor.matmul(
                        scps[h * 64:(h + 1) * 64, g, :],
                        qpT[:, dc, v * S:(v + 1) * S],
                        kpT[:, dc, v * S:(v + 1) * S],
                        start=(dc == 0),
                        stop=(dc == 1),
                        tile_position=(0, h * 64),
                    )

            # ---- softmax along free axis (no max-subtraction: |scores/16|<~10) ----
            attn = sm.tile([128, 4, S], fp32, tag="attn")
            _e = nc.scalar.activation(attn, scps, AF.Exp, scale=float(SCALE))
            # keep Exp-set ops contiguous on ACT: exp of a new gelu-group must
            # come after the previous group's last gelu
            if last_gelu is not None:
                add_dep_helper(_e.ins, last_gelu, sync=False,
                               reason="act-table grouping: exp after prior gelus")
            last_exp = _e.ins
            sums = sm.tile([128, 4], fp32, tag="sums")
            nc.vector.tensor_reduce(out=sums, in_=attn, axis=mybir.AxisListType.X, op=ALU.add)
            rec = sm.tile([128, 4], fp32, tag="rec")
            nc.vector.reciprocal(rec, sums)
            nc.vector.tensor_tensor(
                attn, attn, rec[:, :, None].to_broadcast((128, 4, S)), ALU.mult
            )

            if stage <= 3:
                nc.sync.dma_start(r3(out_d[:])[:, 0, st * R: st * R + 256], attn.rearrange("p a b -> p (a b)"))
                continue
            # ---- transpose attn packs; duplicate into both partition halves ----
            atps = psT.tile([128, 4, 128], fp32, tag="attnT")
            for g in range(4):
                for h in range(2):
                    nc.tensor.matmul(
                        atps[h * 64:(h + 1) * 64, g, :],
                        attn[:, g, :],
                        i128,
                        start=True,
                        stop=True,
                        tile_position=(0, h * 64),
                    )
            attnT = sm.tile([128, 4, 128], fp32, tag="attnT_sb")
            nc.scalar.copy(out=attnT, in_=atps)

            if stage <= 4:
                nc.sync.dma_start(r3(out_d[:])[:, 0, st * R: st * R + 512], attnT.rearrange("p a b -> p (a b)"))
                continue
            # ---- attn @ vp, directly in transposed space outT[d, row] ----
            # Concurrent row-group matmuls must not drain into the same
            # (partition, bank) pair: one PSUM tile per row-half h.
            outT = proj.tile([128, 2, R], f32r, tag="outT")
            for c in range(2):
                for h in range(2):
                    pso = psB.tile([128, 4, S], fp32, tag="psB", name="ps_av")
                    for g in range(4):
                        nc.tensor.matmul(
                            pso[:, g, :],
                            vp[h * 64:(h + 1) * 64, g, c * 128:(c + 1) * 128],
                            attnT[h * 64:(h + 1) * 64, g, h * 64:(h + 1) * 64],
                            start=True,
                            stop=True,
                            tile_position=(h * 64, 0),
                        )
                    # view v=2g+h lives at free offset v*64 of outT chunk c
                    o_ap = outT[:, c, :].rearrange(
                        "p (g two s) -> p g two s", two=2, s=S
                    )[:, :, h, :]
                    if c == 0:
                        nc.vector.tensor_copy(o_ap, pso)
                    else:
                        nc.scalar.copy(out=o_ap, in_=pso)

            if stage <= 5:
                nc.sync.dma_start(r3(out_d[:])[:, :, rs], outT)
                continue
            # ---- final projection, staged pre-gelu (Exp and Gelu live in
            # different ACT table sets; group gelus to amortize ~2.7us
            # table switches) ----
            pre = stg.tile([128, 2, R], fp32, tag="pre")
            for mc in range(2):
                psf = psA.tile([128, R], fp32, tag="psA", name="ps_fin")
                for kc in range(2):
                    nc.tensor.matmul(
                        psf,
                        wo[:, kc, mc * 128:(mc + 1) * 128],
                        outT[:, kc, :],
                        start=(kc == 0),
                        stop=(kc == 1),
                    )
                if mc == 0:
                    nc.vector.tensor_copy(pre[:, mc, :], psf)
                else:
                    nc.scalar.copy(out=pre[:, mc, :], in_=psf)
            pending.append((st, pre, qt))

            if len(pending) == GELU_GROUP or st == n_st - 1:
                for pst, ppre, pqt in pending:
                    outsb = proj.tile([128, 2, R], fp32, tag="outsb")
                    for mc in range(2):
                        _g = nc.scalar.activation(
                            out=outsb[:, mc, :], in_=ppre[:, mc, :],
                            func=AF.Gelu, bias=bo_sb[:, mc:mc + 1], scale=1.0,
                        )
                        if last_exp is not None:
                            add_dep_helper(_g.ins, last_exp, sync=False,
                                           reason="act-table grouping: gelu after group exps")
                        last_gelu = _g.ins
                        nc.vector.tensor_add(
                            out=outsb[:, mc, :], in0=outsb[:, mc, :],
                            in1=pqt[:, mc, :],
                        )
                    nc.sync.dma_start(
                        r3(out_d[:])[:, :, pst * R:(pst + 1) * R], outsb
                    )
                pending = []

    nc.finalize()
    return nc


def _get_nc():
    if "nc" not in _CACHE:
        _CACHE["nc"] = _build()
    return _CACHE["nc"]


def _host_inputs(q, k, v, Wq, Wk, Wv, Wo, bo):
    pe = _make_posenc(D, S)                      # [S, D]
    peT_rep = np.ascontiguousarray(np.tile(pe.T, (1, NV)))   # [D, R]
    e2 = np.ascontiguousarray(np.tile(np.eye(S, dtype=np.float32), (1, 2)))
    i128 = np.eye(128, dtype=np.float32)
    consts = {
        "WqT": np.ascontiguousarray(np.asarray(Wq, np.float32).T),
        "WkT": np.ascontiguousarray(np.asarray(Wk, np.float32).T),
        "WvT": np.ascontiguousarray(np.asarray(Wv, np.float32).T),
        "WoT": np.ascontiguousarray(np.asarray(Wo, np.float32).T),
        "bo": np.ascontiguousarray(np.asarray(bo, np.float32)),
        "peT_rep": peT_rep,
        "pe_nat": pe,
        "E2": e2,
        "I128": i128,
    }
    in_maps = []
    for c in range(N_CORES):
        sl = slice(c * VC, (c + 1) * VC)
        m = dict(consts)
        m["qT"] = np.ascontiguousarray(
            np.asarray(q, np.float32)[sl].reshape(ROWS, D).T)
        m["kT"] = np.ascontiguousarray(
            np.asarray(k, np.float32)[sl].reshape(ROWS, D).T)
        m["vT"] = np.ascontiguousarray(
            np.asarray(v, np.float32)[sl].reshape(ROWS, D).T)
        in_maps.append(m)
    return in_maps


def kernel(q, k, v, Wq, Wk, Wv, Wo, bo, _trace=False):
    from concourse.bass_utils import run_bass_kernel_spmd

    nc = _get_nc()
    in_maps = _host_inputs(q, k, v, Wq, Wk, Wv, Wo, bo)
    res = run_bass_kernel_spmd(nc, in_maps, list(range(N_CORES)), trace=_trace)
    outs = [
        res.results[c]["outT"].reshape(D, VC, S).transpose(1, 2, 0)
        for c in range(N_CORES)
    ]
    full = np.concatenate(outs, axis=0)
    if _trace:
        _CACHE["last_results"] = res
    return full



# revision 37
# speedup vs baseline: 2.0102x; 2.0102x over previous
"""Trainium2 Bass kernel for per-view cross-attention (v4, fp16 datapath).

Reference computation (per view v of 1024, S=64 samples, D=256):
  qp = q @ Wq.T + pe ; kp = k @ Wk.T + pe ; vp = v @ Wv.T + pe
  attn = softmax(qp @ kp.T / sqrt(D))
  x = gelu(attn @ vp @ Wo.T + bo) + q
Sharding: data-parallel over the 1024 views across 8 cores (128 views each).

Key optimizations over the fp32 baseline:
- Full fp16 datapath (PSUM accumulation stays fp32): matmuls at 1 cyc/row
  (fp32 pays 4 on sub-256 tiles), DMA traffic halved, DVE 2x/4x modes.
- Host-folded weights: attn@vp@Wo.T = attn @ (v@(Wo@Wv).T + pe@Wo.T),
  removing a whole [rows,256]x[256,256] projection from the PE. Since
  softmax rows sum to 1, bias bo folds into the pe term too, and the
  gelu-approx's 0.5 factor folds into both.
- gelu via tanh approximation using the ACT-table set that also holds
  exp -> zero act-table switches:
    x = 2*x0;  gelu(x) ~= (tanh(2c*x0 + 8ac*x0^3) + 1) * x0
  with a=0.044715, c=sqrt(2/pi), done in tensor_tensor/tensor_scalar ops
  (the only DVE ops with 2x/4x fp16 modes), split per dout-half so the
  pipeline drain is short.
- Softmax fully fp16: exp(x/16 - 6) biased to keep row sums in fp16
  range; the e^-6 factor cancels in the normalization.
- pe-adds folded into matmul accumulation groups via a [64,...] selector
  matmul (keeps DVE/ACT free).
- q/k/v packed in one DRAM tensor -> single input DMA per supertile; all
  weights/tables packed into two DMAs issued from ACT so the first
  supertile load starts immediately on SP.
- One shared 3-deep PSUM ring for all projection chains (q/k/vpw), plus
  scores, transpose, and 3-deep output rings: exactly 8 banks.
- 3-stage software pipeline: projections for supertile st, scores/softmax
  for st-1, attention+output tail for st-2, so no engine waits on a
  just-produced operand.
"""

import sys
import os

for p in ("/opt/trn_rl_repo",):
    if p not in sys.path and os.path.isdir(p):
        sys.path.insert(0, p)

import numpy as np

V, S, D = 1024, 64, 256
N_CORES = 8
VC = V // N_CORES          # views per core
ROWS = VC * S              # 8192 rows per core
R = 512                    # rows per supertile (8 views)
NST = ROWS // R            # supertiles per core
NV = R // S                # views per supertile
SCALE = 1.0 / np.sqrt(np.float32(D)).astype(np.float32)
GA = 0.044715
GC = float(np.sqrt(2.0 / np.pi))
EXP_BIAS = -6.0

QK_EVAC = "act"
VPW_EVAC = "dve"
AT_EVAC = "act"
STG_EVAC = "act"

_CACHE = {}


def _make_posenc(d_hid, n_samples):
    pos = np.arange(n_samples, dtype=np.float64)[:, None]
    j = np.arange(d_hid)[None, :]
    angle = pos / np.power(10000.0, 2.0 * (j // 2) / d_hid)
    table = np.where(j % 2 == 0, np.sin(angle), np.cos(angle))
    return table  # [S, D] float64


def _build(rows=ROWS, stage=99, qk_evac=None, vpw_evac=None, at_evac=None,
           stg_evac=None, psa_bufs=4, pso_bufs=2):
    import concourse.bass as bass
    import concourse.mybir as mybir
    import concourse.tile as tile
    from concourse import bacc
    from contextlib import ExitStack

    qk_evac = qk_evac or QK_EVAC
    vpw_evac = vpw_evac or VPW_EVAC
    at_evac = at_evac or AT_EVAC
    stg_evac = stg_evac or STG_EVAC

    fp32 = mybir.dt.float32
    fp16 = mybir.dt.float16
    AF = mybir.ActivationFunctionType
    ALU = mybir.AluOpType
    n_st = rows // R

    nc = bacc.Bacc(None, target_bir_lowering=False)

    qkv_d = nc.dram_tensor("qkvT", [3 * D, rows], fp16, kind="ExternalInput")
    # packed constants: [128, 2688] = wq(2x256) wk(2x256) w2(2x256) i128(128)
    c128_d = nc.dram_tensor("C128", [128, 1664], fp16, kind="ExternalInput")
    # packed constants: [64, 1024] = peN(256) PW(256) E512(512)
    c64_d = nc.dram_tensor("C64", [S, 1024], fp16, kind="ExternalInput")
    out_d = nc.dram_tensor("outT", [D, rows], fp16, kind="ExternalOutput")

    def r3(ap, p=128):  # [C*p, X] dram -> [p, C, X] partition view
        return ap.rearrange("(c p) r -> p c r", p=p)

    def copy_on(eng, out, in_):
        if eng == "act":
            return nc.scalar.copy(out=out, in_=in_)
        if eng == "dve":
            return nc.vector.tensor_copy(out, in_)
        return nc.gpsimd.tensor_copy(out, in_)

    with tile.TileContext(nc) as tc, ExitStack() as ctx:
        const = ctx.enter_context(tc.tile_pool(name="const", bufs=1))
        ld = ctx.enter_context(tc.tile_pool(name="ld", bufs=6))
        proj = ctx.enter_context(tc.tile_pool(name="proj", bufs=3))
        sm = ctx.enter_context(tc.tile_pool(name="sm", bufs=2))
        tail = ctx.enter_context(tc.tile_pool(name="tail", bufs=4))
        psA = ctx.enter_context(tc.tile_pool(name="psA", bufs=psa_bufs, space="PSUM"))
        psS = ctx.enter_context(tc.tile_pool(name="psS", bufs=1, space="PSUM"))
        psT = ctx.enter_context(tc.tile_pool(name="psT", bufs=1, space="PSUM"))
        psO = ctx.enter_context(tc.tile_pool(name="psO", bufs=pso_bufs, space="PSUM"))

        c128 = const.tile([128, 1664], fp16)
        c64 = const.tile([S, 1024], fp16)
        nc.scalar.dma_start(c128[:, 0:512], c128_d[:][:, 0:512])  # wq first
        nc.scalar.dma_start(c64, c64_d[:])
        nc.scalar.dma_start(c128[:, 512:1664], c128_d[:][:, 512:1664])
        wq = c128[:, 0:512].rearrange("p (c n) -> p c n", c=2)
        wk = c128[:, 512:1024].rearrange("p (c n) -> p c n", c=2)
        w2 = c128[:, 1024:1536].rearrange("p (c n) -> p c n", c=2)
        i128 = c128[:, 1536:1664]
        pen = c64[:, 0:256]
        pw = c64[:, 256:512]
        e5 = c64[:, 512:1024]
        ebias = const.tile([128, 1], fp32)
        nc.vector.memset(ebias[:], EXP_BIAS)

        state = {}

        def qk_chain(qkv, t, w_sb, o_sb, mc):
            ps = psA.tile([128, R], fp32, tag="psA", name="ps_qk")
            for kc in range(2):
                nc.tensor.matmul(
                    ps,
                    w_sb[:, kc, mc * 128:(mc + 1) * 128],
                    qkv[:, t * 2 + kc, :],
                    start=(kc == 0),
                    stop=False,
                )
            nc.tensor.matmul(
                ps, pen[:, mc * 128:(mc + 1) * 128], e5,
                start=False, stop=True,
            )
            copy_on(qk_evac, o_sb[:, mc, :], ps)

        def vpw_chain(qkv, vpw, g):
            psf = psA.tile([128, R], fp32, tag="psA", name="ps_vpw")
            ps = psf.rearrange("p (a b) -> p a b", a=2)
            for gg in (g, g + 1):
                for kc in range(2):
                    nc.tensor.matmul(
                        ps[:, gg - g, :],
                        qkv[:, 4 + kc, gg * 128:(gg + 1) * 128],
                        w2[:, kc, :],
                        start=(kc == 0),
                        stop=False,
                    )
                nc.tensor.matmul(
                    ps[:, gg - g, :], e5[:, :128], pw,
                    start=False, stop=True,
                )
            copy_on(vpw_evac, vpw[:, g:g + 2, :], ps)

        def scores_softmax(st):
            qkv, qpT, kpT, vpw, rs = state[st]
            scps = psS.tile([128, 4, S], fp32, tag="scores")
            for v in range(NV):
                g, h = v // 2, v % 2
                for dc in range(2):
                    nc.tensor.matmul(
                        scps[h * 64:(h + 1) * 64, g, :],
                        qpT[:, dc, v * S:(v + 1) * S],
                        kpT[:, dc, v * S:(v + 1) * S],
                        start=(dc == 0),
                        stop=(dc == 1),
                        tile_position=(0, h * 64),
                    )
            # softmax along free axis, fp16 throughout: exp(x/16 - 6) keeps
            # row sums < 64*e^4 ~ 3.5e3 inside fp16 range; the e^-6 factor
            # cancels in the normalization.
            attn = sm.tile([128, 4, S], fp16, tag="attn")
            nc.scalar.activation(attn, scps, AF.Exp, scale=float(SCALE), bias=ebias[:])
            sums = sm.tile([128, 4], fp16, tag="sums")
            rec = sm.tile([128, 4], fp16, tag="rec")
            with nc.allow_low_precision(reason="row sums of <=64 exp terms; fp16 keeps DVE 2x mode"):
                nc.vector.tensor_reduce(out=sums, in_=attn, axis=mybir.AxisListType.X, op=ALU.add)
                nc.vector.reciprocal(rec, sums)
            nc.gpsimd.tensor_tensor(
                attn, attn, rec[:, :, None].to_broadcast((128, 4, S)), ALU.mult
            )
            state[st] = (qkv, rs, attn, vpw)

        def tail_at(p):
            # transpose attn packs; duplicate into both partition halves
            qkv, rs, attn, vpw = state[p]
            atps = psT.tile([128, 4, 128], fp32, tag="attnT")
            for g in range(4):
                for h in range(2):
                    nc.tensor.matmul(
                        atps[h * 64:(h + 1) * 64, g, :],
                        attn[:, g, :],
                        i128,
                        start=True,
                        stop=True,
                        tile_position=(0, h * 64),
                    )
            attnT = sm.tile([128, 4, 128], fp16, tag="attnT_sb")
            copy_on(at_evac, attnT, atps)
            state[p] = (qkv, rs, attnT, vpw)

        def tail_av(p, c):
            # attn @ vpw -> x0 = 0.5*(attn@vp@Wo.T + bo), transposed [dout,row]
            qkv, rs, attnT, vpw = state[p][:4]
            stg = tail.tile([128, 4, 2, S], fp16, tag="stg")
            for h in range(2):
                pso = psO.tile([128, 4, S], fp32, tag="pso", name="ps_av")
                for g in range(4):
                    nc.tensor.matmul(
                        pso[:, g, :],
                        vpw[h * 64:(h + 1) * 64, g, c * 128:(c + 1) * 128],
                        attnT[h * 64:(h + 1) * 64, g, h * 64:(h + 1) * 64],
                        start=True,
                        stop=True,
                        tile_position=(h * 64, 0),
                    )
                copy_on("act" if h == 0 else "dve", stg[:, :, h, :], pso)
            state[p] += ((c, stg),)

        def tail_chain(p, c):
            # x = 2*x0; gelu(x) ~= (tanh(2c*x0 + 8ac*x0^3)+1)*x0; + residual
            ent = state[p]
            qkv, rs = ent[0], ent[1]
            stg = dict(ent[4:])[c]
            x0 = stg.rearrange("p g h s -> p (g h s)")
            vv = tail.tile([128, R], fp16, tag="vv")
            nc.vector.tensor_tensor(vv, x0, x0, ALU.mult)
            nc.vector.tensor_scalar(
                out=vv, in0=vv, scalar1=8.0 * GA * GC, scalar2=2.0 * GC,
                op0=ALU.mult, op1=ALU.add,
            )
            nc.vector.tensor_tensor(vv, vv, x0, ALU.mult)
            th = tail.tile([128, R], fp16, tag="th")
            nc.scalar.activation(th, vv, AF.Tanh, scale=1.0)
            nc.vector.tensor_scalar(
                out=th, in0=th, scalar1=1.0, scalar2=None, op0=ALU.add,
            )
            outsb = tail.tile([128, R], fp16, tag="outsb")
            nc.vector.tensor_tensor(outsb, th, x0, ALU.mult)
            nc.gpsimd.tensor_add(out=outsb, in0=outsb, in1=qkv[:, c, :])
            nc.sync.dma_start(r3(out_d[:])[:, c, rs], outsb)

        def tail_av_fine(p):
            qkv, rs, attnT, vpw = state[p][:4]
            qres = qkv[:, 0:2, :].rearrange("p c (g h s) -> p c g h s", h=2, s=S)
            outv = r3(out_d[:])[:, :, rs].rearrange(
                "p c (g h s) -> p c g h s", h=2, s=S
            )
            for c in range(2):
                for h in range(2):
                    pso = psO.tile([128, 4, S], fp32, tag="pso", name="ps_av")
                    for g in range(4):
                        nc.tensor.matmul(
                            pso[:, g, :],
                            vpw[h * 64:(h + 1) * 64, g, c * 128:(c + 1) * 128],
                            attnT[h * 64:(h + 1) * 64, g, h * 64:(h + 1) * 64],
                            start=True,
                            stop=True,
                            tile_position=(h * 64, 0),
                        )
                    stq = tail.tile([128, 4, S], fp16, tag="stq")
                    copy_on("act" if h == 0 else "dve", stq, pso)
                    x0 = stq.rearrange("p g s -> p (g s)")
                    vv = tail.tile([128, 4 * S], fp16, tag="vvq")
                    nc.vector.tensor_tensor(vv, x0, x0, ALU.mult)
                    nc.vector.tensor_scalar(
                        out=vv, in0=vv, scalar1=8.0 * GA * GC, scalar2=2.0 * GC,
                        op0=ALU.mult, op1=ALU.add,
                    )
                    nc.vector.tensor_tensor(vv, vv, x0, ALU.mult)
                    th = tail.tile([128, 4 * S], fp16, tag="thq")
                    nc.scalar.activation(th, vv, AF.Tanh, scale=1.0)
                    nc.vector.tensor_scalar(
                        out=th, in0=th, scalar1=1.0, scalar2=None, op0=ALU.add,
                    )
                    oq = tail.tile([128, 4, S], fp16, tag="outq")
                    nc.vector.tensor_tensor(
                        oq.rearrange("p g s -> p (g s)"), th, x0, ALU.mult
                    )
                    nc.gpsimd.tensor_add(out=oq, in0=oq, in1=qres[:, c, :, h, :])
                    nc.sync.dma_start(outv[:, c, :, h, :], oq)

        for it in range(n_st):
            st, p1, p2 = it, it - 1, it - 2
            rs = slice(st * R, (st + 1) * R)
            qkv = ld.tile([128, 6, R], fp16, tag="qkv")
            # split loads: q first, then k, then v, so projections start sooner
            nc.sync.dma_start(qkv[:, 0:2, :], r3(qkv_d[:])[:, 0:2, rs])
            nc.sync.dma_start(qkv[:, 2:4, :], r3(qkv_d[:])[:, 2:4, rs])
            nc.sync.dma_start(qkv[:, 4:6, :], r3(qkv_d[:])[:, 4:6, rs])
            qpT = proj.tile([128, 2, R], fp16, tag="qpT")
            kpT = proj.tile([128, 2, R], fp16, tag="kpT")
            vpw = proj.tile([128, 4, D], fp16, tag="vpw")
            state[st] = (qkv, qpT, kpT, vpw, rs)
            if p2 >= 0:
                tail_at(p2)
            qk_chain(qkv, 0, wq, qpT, 0)
            qk_chain(qkv, 0, wq, qpT, 1)
            if p1 >= 0:
                scores_softmax(p1)
            vpw_chain(qkv, vpw, 0)
            p3 = it - 3
            if p3 >= 0:
                tail_chain(p3, 0)
            qk_chain(qkv, 1, wk, kpT, 0)
            qk_chain(qkv, 1, wk, kpT, 1)
            vpw_chain(qkv, vpw, 2)
            if p3 >= 0:
                tail_chain(p3, 1)
            if p2 >= 0:
                tail_av(p2, 0)
                tail_av(p2, 1)
        # drain: last supertile in quarter-granularity pieces to shorten the
        # serial stg->gelu->residual->dma chain
        scores_softmax(n_st - 1)
        tail_chain(n_st - 3, 0)
        tail_at(n_st - 2)
        tail_av(n_st - 2, 0)
        tail_chain(n_st - 3, 1)
        tail_at(n_st - 1)
        tail_av(n_st - 2, 1)
        tail_chain(n_st - 2, 0)
        tail_chain(n_st - 2, 1)
        tail_av_fine(n_st - 1)

    nc.finalize()
    return nc


def _get_nc():
    if "nc" not in _CACHE:
        _CACHE["nc"] = _build()
    return _CACHE["nc"]


def _host_inputs(q, k, v, Wq, Wk, Wv, Wo, bo):
    pe = _make_posenc(D, S)                                # [S, D] f64
    Wq64 = np.asarray(Wq, np.float64)
    Wk64 = np.asarray(Wk, np.float64)
    Wv64 = np.asarray(Wv, np.float64)
    Wo64 = np.asarray(Wo, np.float64)
    bo64 = np.asarray(bo, np.float64)

    def chunk128(wT):  # [256, N] -> [128, 2*N] (partition-chunk packing)
        n = wT.shape[1]
        return wT.reshape(2, 128, n).transpose(1, 0, 2).reshape(128, 2 * n)

    c128 = np.concatenate(
        [
            chunk128(Wq64.T),
            chunk128(Wk64.T),
            chunk128(0.5 * (Wo64 @ Wv64).T),
            np.eye(128),
        ],
        axis=1,
    ).astype(np.float16)
    c64 = np.concatenate(
        [
            pe,                                      # peN [64, 256]
            0.5 * (pe @ Wo64.T + bo64[None, :]),     # PW  [64, 256]
            np.tile(np.eye(S), (1, R // S)),         # E512 [64, 512]
        ],
        axis=1,
    ).astype(np.float16)
    consts = {"C128": c128, "C64": c64}
    q32 = np.asarray(q, np.float32)
    k32 = np.asarray(k, np.float32)
    v32 = np.asarray(v, np.float32)
    in_maps = []
    for c in range(N_CORES):
        sl = slice(c * VC, (c + 1) * VC)
        m = dict(consts)
        m["qkvT"] = np.concatenate(
            [
                q32[sl].reshape(ROWS, D).T,
                k32[sl].reshape(ROWS, D).T,
                v32[sl].reshape(ROWS, D).T,
            ],
            axis=0,
        ).astype(np.float16)
        in_maps.append(m)
    return in_maps


def kernel(q, k, v, Wq, Wk, Wv, Wo, bo, _trace=False):
    from concourse.bass_utils import run_bass_kernel_spmd

    nc = _get_nc()
    in_maps = _host_inputs(q, k, v, Wq, Wk, Wv, Wo, bo)
    res = run_bass_kernel_spmd(nc, in_maps, list(range(N_CORES)), trace=_trace)
    outs = [
        res.results[c]["outT"].astype(np.float32).reshape(D, VC, S).transpose(1, 2, 0)
        for c in range(N_CORES)
    ]
    full = np.concatenate(outs, axis=0)
    if _trace:
        _CACHE["last_results"] = res
    return full
